# revision 30
# baseline (speedup 1.0000x reference)
"""EEGGraphConvNetLSTM on 8 TRN2 NeuronCores (Bass/Tile).

Strategy: graph-level data parallel. Each core gets 16 graphs (1024 nodes)
plus a 64-node halo (previous core's last graph) used to burn in the LSTM
state. GCN message passing is done as dense block-diagonal [128x128]
adjacency matmuls (2 graphs per block). BatchNorm batch statistics are
all-reduced across cores. The 8192-step LSTM is run as 128 parallel chunks
of 8 steps per core, each chunk warmed up with B=32 burn-in steps (forget-
gate decay makes the truncation error ~1e-3).
"""

import numpy as np
from contextlib import ExitStack

import concourse.bass as bass
import concourse.mybir as mybir
from concourse.tile import TileContext
from concourse.bass_utils import run_bass_kernel_spmd
from concourse.vector_clock import ScopedClock

# ---------------- walrus workaround: <=1 sync wait per instruction ----------
import concourse.tile as tile_mod


def _split_all_waits(nc):
    for _, b in list(nc.bb_map.items()):
        insts = b.bb.instructions
        out = []
        changed = False
        for ins in insts:
            si = getattr(ins, "sync_info", None)
            if si is not None and si.on_wait and len(si.on_wait) > 1:
                waits = list(si.on_wait)
                spill, keep = waits[:-1], waits[-1:]
                si.on_wait = keep
                for w in spill:
                    nop = mybir.InstNoOp(
                        name=nc.get_next_instruction_name(), ins=[], outs=[]
                    )
                    nop.engine = ins.engine
                    nop.sync_info = mybir.SyncInfo(on_wait=[w], on_update=[])
                    nc.register_instruction(nop)
                    out.append(nop)
                changed = True
            out.append(ins)
        if changed:
            b.bb.instructions[:] = out


def _patched_drain(self, tick_clock, wait_clock):
    nc = self.nc
    drain = nc.sync.drain()
    wait_clock.add_sem_waits(drain.ins, ScopedClock({None: tick_clock.global_clock}))
    nc.all_engine_barrier()
    assert self.sems is not None
    popped = nc._tile_sem_poison_stack.pop()
    assert popped is self._sem_poison
    nc.clear_and_free_semaphores(list(self.sems.allocated().values()))
    nc.all_engine_barrier()
    _split_all_waits(nc)


tile_mod.TileContext._drain_and_barrier = _patched_drain

# ---------------- constants ----------------
NCORES = 8
G, NPG = 128, 64          # graphs, nodes per graph
GPC = G // NCORES         # 16 graphs per core
NLOC = GPC * NPG          # 1024 own nodes
PAD = 64                  # halo (prev graph) + tail zero pad
NT = NLOC + 2 * PAD       # 1152 node columns per core
NB = NT // 128            # 9 two-graph blocks
LCH = 8                   # chunk length
C = 128                   # chunks per core
BURN = 16                 # LSTM burn-in steps
STEPS = BURN + LCH
JUNK_PER_STEP = 0         # PE warm-keeper matmuls per LSTM step
H = 256
N_NODES = 8192

DT32 = mybir.dt.float32
DT32R = mybir.dt.float32r
DT16 = mybir.dt.float16
AF = mybir.ActivationFunctionType
ALU = mybir.AluOpType

LAYERS = [(1280, 640), (640, 512), (512, 256)]

# gate permutation: torch (i, f, g, o) -> kernel column order (g, i, f, o)
GPERM = np.concatenate([
    np.arange(512, 768), np.arange(0, 256), np.arange(256, 512),
    np.arange(768, 1024),
])

_CACHE = {}


def _build(has_bias=False):
    nc = bass.Bass()
    # ---- dram params (fp16 for everything feeding fp16 matmuls)
    xT = nc.declare_dram_parameter("xT", [1280, NT], DT16, isOutput=False)
    WT = [
        nc.declare_dram_parameter(f"WT{l+1}", [fi, fo], DT16, isOutput=False)
        for l, (fi, fo) in enumerate(LAYERS)
    ]
    AT = nc.declare_dram_parameter("AT", [NB, 128, 128], DT16, isOutput=False)
    gv = [nc.declare_dram_parameter(f"g{l+1}", [128, LAYERS[l][1] // 128], DT32, False) for l in range(3)]
    bev = [nc.declare_dram_parameter(f"be{l+1}", [128, LAYERS[l][1] // 128], DT32, False) for l in range(3)]
    WihT = nc.declare_dram_parameter("WihT", [256, 1024], DT16, isOutput=False)
    WhhT = nc.declare_dram_parameter("WhhT", [256, 1024], DT16, isOutput=False)
    if has_bias:
        bihhB = nc.declare_dram_parameter("bihhB", [128, 1024], DT16, isOutput=False)
    ident = nc.declare_dram_parameter("ident", [128, 128], DT16, isOutput=False)
    masks = nc.declare_dram_parameter("masks", [2, 128, 256], DT32, isOutput=False)
    selG = nc.declare_dram_parameter("selG", [128, GPC], DT16, isOutput=False)
    fW1T = nc.declare_dram_parameter("fW1T", [256, 128], DT16, isOutput=False)
    fW2T = nc.declare_dram_parameter("fW2T", [128, 64], DT16, isOutput=False)
    fW3T = nc.declare_dram_parameter("fW3T", [64, 2], DT16, isOutput=False)
    fb1 = nc.declare_dram_parameter("fb1", [128, 1], DT32, isOutput=False)
    fb2 = nc.declare_dram_parameter("fb2", [64, 1], DT32, isOutput=False)
    fb3 = nc.declare_dram_parameter("fb3", [2, 1], DT32, isOutput=False)
    out_d = nc.declare_dram_parameter("out", [2, GPC], DT32, isOutput=True)

    cc_in = [nc.dram_tensor(f"cc_in{l}", [128, 2 * (LAYERS[l][1] // 128)], DT32) for l in range(3)]
    cc_out = [
        nc.dram_tensor(
            f"cc_out{l}", [NCORES, 128, 2 * (LAYERS[l][1] // 128)], DT32, addr_space="Shared"
        )
        for l in range(3)
    ]
    rg = [list(range(NCORES))]
    cc_wi = nc.dram_tensor("cc_wi", [128, 1], DT32)
    cc_wo = nc.dram_tensor("cc_wo", [NCORES, 128, 1], DT32, addr_space="Shared")

    with TileContext(nc) as tc, ExitStack() as ctx:
        wp = ctx.enter_context(tc.tile_pool(name="wp", bufs=1))
        big = ctx.enter_context(tc.tile_pool(name="big", bufs=1))

        # ---- persistent weight/const tiles
        def load2d(dram, rows, cols, dt, tag, r0=0, c0=0):
            t = wp.tile([rows, cols], dt, tag=tag)
            nc.sync.dma_start(out=t[:], in_=dram[r0 : r0 + rows, c0 : c0 + cols])
            return t

        warm = wp.tile([128, 1], DT32, tag="warm", name="warm")
        nc.vector.memset(warm[:], 0.0)
        psA_cm = tc.tile_pool(name="psA", bufs=1, space="PSUM")
        psA = psA_cm.__enter__()
        xTt = [load2d(xT, 128, NT, DT16, f"xT{k}", r0=k * 128) for k in range(10)]
        # warm the PE through the DMA head: junk matmuls gated on each xT tile
        junkw = psA.tile([128, 512], DT32, tag="junkg", name="junkw")
        for k in range(10):
            for _ in range(3):
                nc.tensor.matmul(
                    junkw[:], lhsT=xTt[k][:, 0:128], rhs=xTt[k][:, 0:512],
                    start=True, stop=True,
                )
        WTt = []
        for l, (fi, fo) in enumerate(LAYERS):
            WTt.append([load2d(WT[l], 128, fo, DT16, f"WT{l}_{k}", r0=k * 128) for k in range(fi // 128)])
        ATt = []
        for b in range(NB):
            t = wp.tile([128, 128], DT16, tag=f"AT{b}", name=f"AT{b}")
            nc.sync.dma_start(out=t[:], in_=AT[b, :, :])
            ATt.append(t)
        WihRt = [load2d(WihT, 128, 1024, DT16, f"WihR{k}", r0=k * 128) for k in range(2)]
        WhhRt = [load2d(WhhT, 128, 1024, DT16, f"WhhR{k}", r0=k * 128) for k in range(2)]
        if has_bias:
            bihhBt = load2d(bihhB, 128, 1024, DT16, "bihhB")
            inv128 = wp.tile([128, 128], DT16, tag="inv128", name="inv128")
            nc.vector.memset(inv128[:], 1.0 / 128.0)
        idt = load2d(ident, 128, 128, DT16, "ident")
        selGt = load2d(selG, 128, GPC, DT16, "selG")
        # per-feature g/be as [128, nft]
        gT, beT = [], []
        for l, (fi, fo) in enumerate(LAYERS):
            nft = fo // 128
            tg = wp.tile([128, nft], DT32, tag=f"gT{l}", name=f"gT{l}")
            tb = wp.tile([128, nft], DT32, tag=f"beT{l}", name=f"beT{l}")
            nc.sync.dma_start(out=tg[:], in_=gv[l][:, :])
            nc.sync.dma_start(out=tb[:], in_=bev[l][:, :])
            gT.append(tg)
            beT.append(tb)
        maskPt = []
        for i in range(2):
            m32 = wp.tile([128, 256], DT32, tag=f"mP{i}", name=f"mP{i}")
            nc.sync.dma_start(out=m32[:], in_=masks[i, :, :])
            maskPt.append(m32)
        fW1Tt = [load2d(fW1T, 128, 128, DT16, f"fW1T{k}", r0=k * 128) for k in range(2)]
        fW2Tt = load2d(fW2T, 128, 64, DT16, "fW2T")
        fW3Tt = load2d(fW3T, 64, 2, DT16, "fW3T")
        fb1t = wp.tile([128, 1], DT32, tag="fb1", name="fb1")
        nc.sync.dma_start(out=fb1t[:], in_=fb1[:, :])
        fb2t = wp.tile([64, 1], DT32, tag="fb2", name="fb2")
        nc.sync.dma_start(out=fb2t[:], in_=fb2[:, :])
        fb3t = wp.tile([2, 1], DT32, tag="fb3", name="fb3")
        epst = wp.tile([128, 1], DT32, tag="epst", name="epst")
        nc.vector.memset(epst[:], 1e-5)
        nc.sync.dma_start(out=fb3t[:], in_=fb3[:, :])

        # ---------------- GCN layers ----------------
        hT = xTt
        for l, (fi, fo) in enumerate(LAYERS):
            K = fi // 128
            nft = fo // 128
            # lin: m[node, fo] node-major, fp16
            m16t = [big.tile([128, 640], DT16, tag=f"m16_{b}", name=f"m16_{b}") for b in range(NB)]
            for nt in range(NB):
                ps = psA.tile([128, 1024], DT32, tag="linps", name="linps", bufs=2)
                if fo == 640:
                    chunks = [(0, 0, 320), (320, 512, 320)]  # (m-col, psum-col, width)
                elif fo == 512:
                    chunks = [(0, 0, 512)]
                else:
                    chunks = [(0, 0, 256)]
                for k in range(K):
                    for (mc, pc, w) in chunks:
                        nc.tensor.matmul(
                            ps[:, pc : pc + w],
                            lhsT=hT[k][:, nt * 128 : (nt + 1) * 128],
                            rhs=WTt[l][k][:, mc : mc + w],
                            start=(k == 0),
                            stop=(k == K - 1),
                        )
                for (mc, pc, w) in chunks:
                    nc.vector.tensor_copy(m16t[nt][:, mc : mc + w], ps[:, pc : pc + w])
            if l == 0:
                # warmup collective mid-L1 so the CC stream is hot for the
                # first stats AllGather (depends on the first lin block)
                nc.vector.tensor_copy(warm[:], m16t[0][:, 0:1])
                nc.sync.dma_start(out=cc_wi[:], in_=warm[:])
                nc.gpsimd.collective_compute(
                    "AllGather", ALU.bypass, replica_groups=rg,
                    ins=[cc_wi[:]], outs=[cc_wo[:, :, :]])
            # scatter: s.T[f, dst] feature-major fp32 + stats
            sT = [big.tile([128, NT], DT32, tag=f"sT{ft}", name=f"sT{ft}") for ft in range(nft)]
            stats = big.tile([128, 2 * nft], DT32, tag=f"stats{l}", name=f"stats{l}")
            sqs = big.tile([128, NLOC], DT32, tag="sqscratch", name="sqscratch")
            sqsv = big.tile([128, NLOC], DT32, tag="sqscratchv", name="sqscratchv")
            for ft in range(nft):
                pss = psA.tile([128, NT], DT32, tag="scps", name="scps")
                for b in range(NB):
                    nc.tensor.matmul(
                        pss[:, b * 128 : (b + 1) * 128],
                        lhsT=m16t[b][:, ft * 128 : (ft + 1) * 128],
                        rhs=ATt[b][:],
                        start=(b % 4 == 0),
                        stop=(b in (3, 7, 8)),
                    )
                nc.scalar.activation(sT[ft][:, 0:PAD], pss[:, 0:PAD], AF.Copy)
                nc.scalar.activation(
                    sT[ft][:, PAD:NT], pss[:, PAD:NT], AF.Copy,
                    accum_out=stats[:, ft : ft + 1],
                )
            # square+sumsq: odd fts on vector (square then reduce), even on scalar
            for ft in range(nft):
                if ft % 2 == 0:
                    nc.scalar.activation(
                        sqs[:], sT[ft][:, PAD : PAD + NLOC], AF.Square,
                        accum_out=stats[:, nft + ft : nft + ft + 1],
                    )
                else:
                    nc.vector.tensor_mul(
                        sqsv[:], sT[ft][:, PAD : PAD + NLOC], sT[ft][:, PAD : PAD + NLOC]
                    )
                    nc.vector.tensor_reduce(
                        stats[:, nft + ft : nft + ft + 1],
                        sqsv[:],
                        axis=mybir.AxisListType.X,
                        op=ALU.add,
                    )
            # allgather stats (lower latency than AllReduce), then local sum
            nc.sync.dma_start(out=cc_in[l][:], in_=stats[:])
            nc.gpsimd.collective_compute(
                "AllGather", ALU.bypass, replica_groups=rg,
                ins=[cc_in[l][:]], outs=[cc_out[l][:, :, :]],
            )
            statsg8 = big.tile(
                [128, NCORES * 2 * nft], DT32, tag=f"statsg8{l}", name=f"statsg8{l}"
            )
            nc.sync.dma_start(
                out=statsg8[:],
                in_=cc_out[l][:, :, :].rearrange("r p f -> p r f"),
            )
            statsg = big.tile([128, 2 * nft], DT32, tag=f"statsg{l}", name=f"statsg{l}")
            nc.vector.tensor_reduce(
                statsg[:],
                statsg8[:].rearrange("p (r f) -> p f r", r=NCORES, f=2 * nft),
                axis=mybir.AxisListType.X,
                op=ALU.add,
            )
            # warm-keeper: PE re-throttles to 1.2GHz during the collective
            # stall; these depend on the AG result so they run right after it,
            # re-warming the PE while the BN scale/bias math runs.
            junkg = psA.tile([128, 512], DT32, tag="junkg", name="junkg")
            for _ in range(16):
                nc.tensor.matmul(
                    junkg[0 : 2 * nft, 0 : NCORES * 2 * nft],
                    lhsT=statsg8[:, 0 : 2 * nft], rhs=statsg8[:],
                    start=True, stop=True,
                )
            # scale/bias
            mu = big.tile([128, nft], DT32, tag="mu", name="mu")
            var = big.tile([128, nft], DT32, tag="var", name="var")
            scl = big.tile([128, nft], DT32, tag="scl", name="scl")
            bia = big.tile([128, nft], DT32, tag="bia", name="bia")
            nc.vector.tensor_scalar_mul(mu[:], statsg[:, 0:nft], 1.0 / N_NODES)
            nc.vector.tensor_scalar_mul(var[:], statsg[:, nft : 2 * nft], 1.0 / N_NODES)
            nc.vector.tensor_mul(scl[:], mu[:], mu[:])
            nc.vector.tensor_sub(var[:], var[:], scl[:])
            nc.scalar.activation(var[:], var[:], AF.Sqrt, bias=epst[:])
            nc.vector.reciprocal(var[:], var[:])
            nc.vector.tensor_mul(scl[:], gT[l][:], var[:])
            nc.vector.tensor_mul(mu[:], mu[:], scl[:])
            nc.vector.tensor_sub(bia[:], beT[l][:], mu[:])
            # apply + leaky -> next hT (fp16, feature-major)
            hTn = [big.tile([128, NT], DT16, tag=f"hT{l}_{ft}", name=f"hT{l}_{ft}") for ft in range(nft)]
            for ft in range(nft):
                nc.scalar.activation(
                    hTn[ft][:], sT[ft][:], AF.Lrelu,
                    bias=bia[:, ft : ft + 1], scale=scl[:, ft : ft + 1], alpha=0.01,
                )
            hT = hTn

        psA_cm.__exit__(None, None, None)

        # ---------------- LSTM (transposed: chunks on partitions, gates on cols)
        # psum P[c, g] = sum_f Wih[g, f] h3[f, node(c, t)] + sum_j Whh[g, j] h[j, c]
        # via lhsT = hT3 strided / h_sbT, rhs = WihR / WhhR [feat, 1024 gates].
        lsp = ctx.enter_context(tc.tile_pool(name="lsp", bufs=2))
        one = ctx.enter_context(tc.tile_pool(name="one", bufs=1))
        h_sbT = [one.tile([128, 128], DT16, tag=f"h_sbT{k}", name=f"h_sbT{k}") for k in range(2)]
        c_sb = one.tile([128, 256], DT32, tag="c_sb", name="c_sb")
        acc = one.tile([128, 256], DT32, tag="acc", name="acc")
        for k in range(2):
            nc.vector.memset(h_sbT[k][:], 0.0)
        nc.vector.memset(c_sb[:], 0.0)
        nc.vector.memset(acc[:], 0.0)
        psB_cm = tc.tile_pool(name="psB", bufs=1, space="PSUM")
        psB = psB_cm.__enter__()
        mstep = tuple(BURN - 1 - i * LCH for i in range(2) if BURN - 1 - i * LCH >= 0)
        junk = psB.tile([128, 512], DT32, tag="junk", name="junk")
        # gate column order is [g, i | f, o] (host permutes Wih/Whh rows):
        # PB holds (g, i) so the tanh(g) -> i*g path starts as soon as its
        # Whh matmuls land; PA holds (f, o) in a SEPARATE psum tile so its
        # matmuls don't false-depend on the PB sigmoid reads.
        for t in range(STEPS):
            off = PAD - BURN + t
            PB = psB.tile([128, 512], DT32, tag="PB", name="PB", bufs=2)
            PA = psB.tile([128, 512], DT32, tag="PA", name="PA", bufs=2)
            sg = lsp.tile([128, 1024], DT16, tag="sg", name="sg")
            # state-independent Wih matmuls first (fill PE while prev step's
            # gate math runs), then warm-keeper dummies, then Whh
            for P, cs in ((PB, slice(0, 512)), (PA, slice(512, 1024))):
                for k in range(2):
                    nc.tensor.matmul(
                        P[:],
                        lhsT=hT[k][:, off : off + C * LCH : LCH],
                        rhs=WihRt[k][:, cs],
                        start=(k == 0),
                        stop=False,
                    )
                if has_bias:
                    nc.tensor.matmul(
                        P[:], lhsT=inv128[:], rhs=bihhBt[:, cs],
                        start=False, stop=False,
                    )
            for _ in range(JUNK_PER_STEP):
                nc.tensor.matmul(junk[:], lhsT=idt[:], rhs=WhhRt[0][:, 0:512],
                                 start=True, stop=True)
            for P, cs in ((PB, slice(0, 512)), (PA, slice(512, 1024))):
                for k in range(2):
                    nc.tensor.matmul(
                        P[:],
                        lhsT=h_sbT[k][:],
                        rhs=WhhRt[k][:, cs],
                        start=False,
                        stop=(k == 1),
                    )
            nc.scalar.activation(sg[:, 0:256], PB[:, 0:256], AF.Tanh)       # g
            nc.scalar.activation(sg[:, 256:512], PB[:, 256:512], AF.Sigmoid)  # i
            nc.scalar.activation(sg[:, 512:1024], PA[:], AF.Sigmoid)        # f, o
            t1 = lsp.tile([128, 256], DT32, tag="t1", name="t1")
            t2 = lsp.tile([128, 256], DT16, tag="t2", name="t2")
            th = lsp.tile([128, 256], DT16, tag="th", name="th")
            tho = lsp.tile([128, 256], DT16, tag="tho", name="tho")
            nc.vector.tensor_mul(t2[:], sg[:, 256:512], sg[:, 0:256])  # i*g
            nc.vector.tensor_mul(t1[:], sg[:, 512:768], c_sb[:])      # f*c
            nc.vector.tensor_add(c_sb[:], t1[:], t2[:])
            nc.scalar.activation(th[:], c_sb[:], AF.Tanh)
            nc.vector.tensor_mul(tho[:], th[:], sg[:, 768:1024])  # o*tanh(c)
            if t >= BURN:
                nc.vector.tensor_add(acc[:], acc[:], tho[:])
            if t in mstep:
                mi = (BURN - 1 - t) // LCH
                nc.vector.tensor_mul(tho[:], tho[:], maskPt[mi][:])
                nc.vector.tensor_mul(c_sb[:], c_sb[:], maskPt[mi][:])
            # transpose tho [chunk, hid] -> h_sbT [hid, chunk]
            for k in range(2):
                pT = psB.tile([128, 128], DT32, tag=f"pT{k}", name=f"pT{k}")
                nc.tensor.matmul(
                    pT[:], lhsT=tho[:, 128 * k : 128 * (k + 1)], rhs=idt[:],
                    start=True, stop=True,
                )
                (nc.scalar.activation(h_sbT[k][:], pT[:], AF.Copy)
                 if k == 0 else nc.vector.tensor_copy(h_sbT[k][:], pT[:]))

        psB_cm.__exit__(None, None, None)

        # ---------------- pool + FC ----------------
        psC_cm = tc.tile_pool(name="psC", bufs=1, space="PSUM")
        psC = psC_cm.__enter__()
        acc16 = one.tile([128, 256], DT16, tag="acc16", name="acc16")
        nc.vector.tensor_copy(acc16[:], acc[:])
        poolPs = psC.tile([GPC, 256], DT32, tag="poolPs", name="poolPs")
        nc.tensor.matmul(poolPs[:], lhsT=selGt[:], rhs=acc16[:], start=True, stop=True)
        pool16 = one.tile([GPC, 256], DT16, tag="pool16", name="pool16")
        nc.vector.tensor_copy(pool16[:], poolPs[:])
        poolT = []
        for k in range(2):
            pTp = psC.tile([128, GPC], DT32, tag=f"pTp{k}", name=f"pTp{k}")
            nc.tensor.matmul(
                pTp[:], lhsT=pool16[:, 128 * k : 128 * (k + 1)],
                rhs=idt[0:GPC, 0:GPC], start=True, stop=True,
            )
            tpt = one.tile([128, GPC], DT16, tag=f"poolT{k}", name=f"poolT{k}")
            nc.vector.tensor_copy(tpt[:], pTp[:])
            poolT.append(tpt)
        fps = psC.tile([128, GPC], DT32, tag="fcps", name="fcps")
        for k in range(2):
            nc.tensor.matmul(fps[:], lhsT=fW1Tt[k][:], rhs=poolT[k][:], start=(k == 0), stop=(k == 1))
        fc1 = one.tile([128, GPC], DT16, tag="fc1", name="fc1")
        nc.scalar.activation(fc1[:], fps[:], AF.Lrelu, bias=fb1t[:], alpha=0.01)
        fps2 = psC.tile([64, GPC], DT32, tag="fcps2", name="fcps2")
        nc.tensor.matmul(fps2[:], lhsT=fW2Tt[:], rhs=fc1[:], start=True, stop=True)
        fc2 = one.tile([64, GPC], DT16, tag="fc2", name="fc2")
        nc.scalar.activation(fc2[:], fps2[:], AF.Lrelu, bias=fb2t[:], alpha=0.01)
        fps3 = psC.tile([2, GPC], DT32, tag="fcps3", name="fcps3")
        nc.tensor.matmul(fps3[:], lhsT=fW3Tt[:], rhs=fc2[:], start=True, stop=True)
        fc3 = one.tile([2, GPC], DT32, tag="fc3", name="fc3")
        nc.scalar.activation(fc3[:], fps3[:], AF.Lrelu, bias=fb3t[:], alpha=0.01)
        nc.sync.dma_start(out=out_d[:], in_=fc3[:])
        psC_cm.__exit__(None, None, None)

    return nc


def _prep_core(inputs, k, A):
    f16 = np.float16
    x = inputs["x"]
    lo, hi = k * NLOC - PAD, k * NLOC + NLOC
    xTk = np.zeros((1280, NT), f16)
    if k == 0:
        xTk[:, PAD : PAD + NLOC] = x[0:NLOC].T
    else:
        xTk[:, 0 : PAD + NLOC] = x[lo:hi].T
    ATk = np.zeros((NB, 128, 128), f16)
    glist = ([-1] if k == 0 else [k * GPC - 1]) + list(range(k * GPC, (k + 1) * GPC)) + [-1]
    for b in range(NB):
        ga, gb = glist[2 * b], glist[2 * b + 1]
        if ga >= 0:
            ATk[b, 0:64, 0:64] = A[ga].T
        if gb >= 0:
            ATk[b, 64:128, 64:128] = A[gb].T
    # per-chunk masks [2, 128 chunk, 256]: mask i zeroes chunk i's state (core 0)
    mk = np.ones((2, 128, 256), np.float32)
    if k == 0:
        for c in range(2):
            if BURN - 1 - c * LCH >= 0:
                mk[c, c, :] = 0.0
    selG = np.zeros((128, GPC), f16)
    for c in range(128):
        selG[c, c // (NPG // LCH)] = 1.0
    bihh = (inputs["bih"] + inputs["bhh"]).astype(np.float32)
    im = {
        "xT": xTk,
        "WT1": inputs["W1"].T.astype(f16).copy(),
        "WT2": inputs["W2"].T.astype(f16).copy(),
        "WT3": inputs["W3"].T.astype(f16).copy(),
        "AT": ATk,
        "g1": inputs["g1"].astype(np.float32).reshape(5, 128).T.copy(),
        "g2": inputs["g2"].astype(np.float32).reshape(4, 128).T.copy(),
        "g3": inputs["g3"].astype(np.float32).reshape(2, 128).T.copy(),
        "be1": inputs["be1"].astype(np.float32).reshape(5, 128).T.copy(),
        "be2": inputs["be2"].astype(np.float32).reshape(4, 128).T.copy(),
        "be3": inputs["be3"].astype(np.float32).reshape(2, 128).T.copy(),
        # gate columns reordered i,f,g,o -> g,i,f,o (kernel layout)
        "WihT": inputs["Wih"].T.astype(f16)[:, GPERM].copy(),
        "WhhT": inputs["Whh"].T.astype(f16)[:, GPERM].copy(),
        "ident": np.eye(128, dtype=f16),
        "masks": mk,
        "selG": selG,
        "fW1T": inputs["fW1"].T.astype(f16).copy(),
        "fW2T": inputs["fW2"].T.astype(f16).copy(),
        "fW3T": inputs["fW3"].T.astype(f16).copy(),
        "fb1": inputs["fb1"].astype(np.float32).reshape(128, 1),
        "fb2": inputs["fb2"].astype(np.float32).reshape(64, 1),
        "fb3": inputs["fb3"].astype(np.float32).reshape(2, 1),
    }
    if np.any(bihh != 0):
        im["bihhB"] = np.repeat(bihh[GPERM][None, :], 128, axis=0).astype(f16)
    return im


def kernel(**inputs):
    inputs = {k: np.asarray(v) for k, v in inputs.items()}
    src, dst = inputs["edge_index"][0], inputs["edge_index"][1]
    ew = inputs["edge_weight"].astype(np.float32)
    A = np.zeros((G, NPG, NPG), np.float32)
    np.add.at(A, (src // NPG, dst % NPG, src % NPG), ew)
    has_bias = bool(np.any(inputs["bih"] + inputs["bhh"] != 0))
    key = f"nc{has_bias}"
    if key not in _CACHE:
        _CACHE[key] = _build(has_bias)
    nc = _CACHE[key]
    in_maps = [_prep_core(inputs, k, A) for k in range(NCORES)]
    res = run_bass_kernel_spmd(nc, in_maps, core_ids=list(range(NCORES)), **_CACHE.get("kw", {}))
    _CACHE["last"] = res
    out = np.zeros((G, 2), np.float32)
    for k in range(NCORES):
        out[k * GPC : (k + 1) * GPC, :] = res.results[k]["out"].T
    return out

